# revision 6
# baseline (speedup 1.0000x reference)
"""DeepSeek sparse attention (MLA + YaRN RoPE + local/dilated/global mask) on 8 TRN2 cores.

Sharding: (batch, head-group) across 8 cores — core c handles batch c//4, heads
[4*(c%4), 4*(c%4)+4).  Each core computes its projections from the full x (host
pre-transposes x per batch), runs block-sparse attention for its 4 heads, and
produces a row-parallel partial of out @ w_o.  Host sums the 4 partials per batch.

Layout: "transposed" activations [feature, t] so every matmul keeps the moving
operand in the free dim (N=512/256) at full bf16 rate and no on-chip transposes
are needed anywhere.  Scores are computed as S^T[k, q]; the softmax denominator
is obtained with an all-ones stationary operand (broadcast across partitions),
so the divide is a plain elementwise mul by the reciprocal.
"""

import sys

if "/opt/trn_rl_repo" not in sys.path:
    sys.path.insert(0, "/opt/trn_rl_repo")

import ml_dtypes
import numpy as np

import concourse.bass as bass  # noqa: F401  (bass types used via tile/bacc)
import concourse.mybir as mybir
import concourse.tile as tile
from concourse import bacc, bass_utils

BF16 = ml_dtypes.bfloat16

# ---- problem constants (hardcoded per contract) ----
B, T, D = 2, 2048, 2048
H, DH, DR, DL = 16, 128, 64, 512
WINDOW, STRIDE, GLOB = 512, 64, 128
BASE, MAX_SEQ, ORIG_MAX = 10000.0, 131072, 4096
BETA_FAST, BETA_SLOW = 32.0, 1.0
SCALE = 1.0 / float(np.sqrt(DH))
SCALE_ROPE = 1.0 / float(np.sqrt(DR))
YARN = float(np.float32(0.1 * np.log(MAX_SEQ / ORIG_MAX) + 1.0))
HALF = WINDOW // 2

NCORES = 8
NH = 4            # heads per core
NP = 2            # head-pairs per core (rope tiles stack 2 heads on 128 partitions)
QB = 256          # query block
NB = T // QB      # 8
SL = 512          # t-slice width in projection phase
NS = T // SL      # 4
NT = T // 128     # 16


def _inv_freq():
    base_inv = 1.0 / (BASE ** (np.arange(0, DR, 2, dtype=np.float32) / DR))
    scale = MAX_SEQ / ORIG_MAX
    freqs = np.arange(DR // 2, dtype=np.float32)
    ramp = np.clip((freqs - BETA_SLOW) / (BETA_FAST - BETA_SLOW), 0.0, 1.0)
    return (base_inv * (1 - ramp) + (base_inv / scale) * ramp).astype(np.float32)


def _full_mask():
    pos = np.arange(T)
    qp, kp = pos[:, None], pos[None, :]
    dist = qp - kp
    window = (dist >= -HALF) & (dist <= HALF)
    dil = (kp % STRIDE == 0) | (kp < GLOB)
    return (window | dil) & (kp <= qp)


def _mask_tiles():
    """Per q-block key tiles, with exactly-once ownership masks.

    Tiles: A0 = keys [0, 128) (global), A1 = 32 dilated keys {64j}, B0..B3 =
    the 512-wide sliding window strip.  A0 owns k<128; A1 owns k%64==0 & k>=128;
    B owns the rest.  Each tile classified: 'skip' (all-zero), 'ones', 'mask'.
    """
    full = _full_mask()
    blocks = []
    for qb in range(NB):
        q0 = qb * QB
        k0 = max(0, q0 - HALF)
        qs = slice(q0, q0 + QB)
        blk = []
        m = full[qs, 0:GLOB].T.copy()                        # [128, QB]
        blk.append(dict(kind="A0", i=0, k0=0, keys=np.arange(GLOB), m=m))
        keys = np.arange(32) * STRIDE
        m = full[qs, :][:, keys].T.copy()                    # [32, QB]
        m[keys < GLOB] = False
        blk.append(dict(kind="A1", i=0, k0=0, keys=keys, m=m))
        for i in range(4):
            kk = k0 + 128 * i + np.arange(128)
            m = full[qs, :][:, kk].T.copy()
            m[(kk < GLOB) | (kk % STRIDE == 0)] = False
            blk.append(dict(kind="B", i=i, k0=k0, keys=kk, m=m))
        for t_ in blk:
            t_["cls"] = ("skip" if not t_["m"].any()
                         else "ones" if t_["m"].all() else "mask")
        blocks.append(blk)
    # exactly-once coverage check against the reference mask
    for qb in range(NB):
        cov = np.zeros((QB, T), dtype=np.int32)
        for t_ in blocks[qb]:
            cov[np.arange(QB)[:, None], t_["keys"][None, :]] += t_["m"].T
        assert (cov == full[qb * QB:(qb + 1) * QB].astype(np.int32)).all()
    return blocks


_MASK_TILES = _mask_tiles()


def _build_program():
    nc = bacc.Bacc("TRN2", target_bir_lowering=False, debug=False,
                   enable_asserts=False, num_devices=NCORES)
    bf, f32 = mybir.dt.bfloat16, mybir.dt.float32

    xT = nc.dram_tensor("xT", [D, T], bf, kind="ExternalInput").ap()
    w_q = nc.dram_tensor("w_q", [D, NH * DH], bf, kind="ExternalInput").ap()
    w_dkv = nc.dram_tensor("w_dkv", [D, DL], bf, kind="ExternalInput").ap()
    w_uk = nc.dram_tensor("w_uk", [DL, NH * DH], bf, kind="ExternalInput").ap()
    w_uv = nc.dram_tensor("w_uv", [DL, NH * DH], bf, kind="ExternalInput").ap()
    w_qp = nc.dram_tensor("w_qp", [D, NH * DR], bf, kind="ExternalInput").ap()
    w_kp = nc.dram_tensor("w_kp", [D, NH * DR], bf, kind="ExternalInput").ap()
    w_o = nc.dram_tensor("w_o", [NH * DH, D], bf, kind="ExternalInput").ap()
    cosT = nc.dram_tensor("cosT", [128, T], f32, kind="ExternalInput").ap()
    ssgnT = nc.dram_tensor("ssgnT", [128, T], f32, kind="ExternalInput").ap()
    mA0 = nc.dram_tensor("mA0", [GLOB, QB], bf, kind="ExternalInput").ap()
    mA1 = nc.dram_tensor("mA1", [NB, 32, QB], bf, kind="ExternalInput").ap()
    mB = nc.dram_tensor("mB", [NB, 4, 128, QB], bf, kind="ExternalInput").ap()
    out = nc.dram_tensor("out", [T, D], f32, kind="ExternalOutput").ap()

    EXP = mybir.ActivationFunctionType.Exp

    with tile.TileContext(nc) as tc:
        with tc.tile_pool(name="acts", bufs=1) as acts, \
             tc.tile_pool(name="consts", bufs=1) as consts:
            q_cT = [acts.tile([128, T], bf, tag=f"q_cT{h}", name=f"q_cT{h}") for h in range(NH)]
            k_cT = [acts.tile([128, T], bf, tag=f"k_cT{h}", name=f"k_cT{h}") for h in range(NH)]
            q_rT = [acts.tile([128, T], bf, tag=f"q_rT{p}", name=f"q_rT{p}") for p in range(NP)]
            k_rT = [acts.tile([128, T], bf, tag=f"k_rT{p}", name=f"k_rT{p}") for p in range(NP)]
            v_t = [acts.tile([128, NH * DH], bf, tag=f"v{t_}", name=f"v{t_}") for t_ in range(NT)]
            v_A = acts.tile([32, NH * DH], bf, tag="v_A", name="v_A")
            att = [acts.tile([128, T], bf, tag=f"att{h}", name=f"att{h}") for h in range(NH)]
            ones = consts.tile([128, 128], bf, tag="ones", name="ones")
            nc.vector.memset(ones, 1.0)

            # ---------------- P1: projections (t-slice streamed) --------------
            with tc.tile_pool(name="wp1", bufs=1) as wp1, \
                 tc.tile_pool(name="xs", bufs=1) as xs, \
                 tc.tile_pool(name="ckvp", bufs=2) as ckvp, \
                 tc.tile_pool(name="rope_t", bufs=3) as rope_t, \
                 tc.tile_pool(name="p1ps", bufs=6, space="PSUM") as p1ps, \
                 tc.tile_pool(name="vAps", bufs=2, space="PSUM") as vAps:
                cos_sb = wp1.tile([128, T], f32, tag="cos", name="cos")
                ssg_sb = wp1.tile([128, T], f32, tag="ssg", name="ssg")
                nc.sync.dma_start(out=cos_sb, in_=cosT)
                nc.sync.dma_start(out=ssg_sb, in_=ssgnT)
                wq_sb = [wp1.tile([128, NH * DH], bf, tag=f"wq{d}", name=f"wq{d}") for d in range(16)]
                wdkv_sb = [wp1.tile([128, DL], bf, tag=f"wdkv{d}", name=f"wdkv{d}") for d in range(16)]
                wqp_sb = [wp1.tile([128, NH * DR], bf, tag=f"wqp{d}", name=f"wqp{d}") for d in range(16)]
                wkp_sb = [wp1.tile([128, NH * DR], bf, tag=f"wkp{d}", name=f"wkp{d}") for d in range(16)]
                wuk_sb = [wp1.tile([128, NH * DH], bf, tag=f"wuk{g}", name=f"wuk{g}") for g in range(4)]
                wuv_sb = [wp1.tile([128, NH * DH], bf, tag=f"wuv{g}", name=f"wuv{g}") for g in range(4)]
                for d in range(16):
                    sl = slice(128 * d, 128 * (d + 1))
                    nc.sync.dma_start(out=wq_sb[d], in_=w_q[sl, :])
                    nc.sync.dma_start(out=wdkv_sb[d], in_=w_dkv[sl, :])
                    nc.sync.dma_start(out=wqp_sb[d], in_=w_qp[sl, :])
                    nc.sync.dma_start(out=wkp_sb[d], in_=w_kp[sl, :])
                for g in range(4):
                    sl = slice(128 * g, 128 * (g + 1))
                    nc.sync.dma_start(out=wuk_sb[g], in_=w_uk[sl, :])
                    nc.sync.dma_start(out=wuv_sb[g], in_=w_uv[sl, :])

                for s in range(NS):
                    t0 = s * SL
                    tsl = slice(t0, t0 + SL)
                    xt = [xs.tile([128, SL], bf, tag=f"xt{d}", name=f"xt{d}") for d in range(16)]
                    for d in range(16):
                        nc.sync.dma_start(
                            out=xt[d], in_=xT[128 * d:128 * (d + 1), tsl])
                    # content q projections
                    for h in range(NH):
                        ps = p1ps.tile([128, SL], f32, tag="proj", name="proj")
                        for d in range(16):
                            nc.tensor.matmul(ps, wq_sb[d][:, h * DH:(h + 1) * DH],
                                             xt[d], start=(d == 0), stop=(d == 15))
                        nc.vector.tensor_copy(out=q_cT[h][:, tsl], in_=ps)
                    # latent kv
                    ckv = [ckvp.tile([128, SL], bf, tag=f"ckv{g}", name=f"ckv{g}") for g in range(4)]
                    for g in range(4):
                        ps = p1ps.tile([128, SL], f32, tag="proj", name="proj")
                        for d in range(16):
                            nc.tensor.matmul(ps, wdkv_sb[d][:, g * 128:(g + 1) * 128],
                                             xt[d], start=(d == 0), stop=(d == 15))
                        nc.vector.tensor_copy(out=ckv[g], in_=ps)
                    # rope projections + rotation (pair-stacked: 2 heads / tile)
                    for w_sb, dstT in ((wqp_sb, q_rT), (wkp_sb, k_rT)):
                        for p in range(NP):
                            ps = p1ps.tile([128, SL], f32, tag="proj", name="proj")
                            for d in range(16):
                                nc.tensor.matmul(ps, w_sb[d][:, p * 128:(p + 1) * 128],
                                                 xt[d], start=(d == 0), stop=(d == 15))
                            m1 = rope_t.tile([128, SL], bf, tag="m1", name="m1")
                            nc.vector.tensor_mul(m1, ps, cos_sb[:, tsl])
                            m2 = rope_t.tile([128, SL], bf, tag="m2", name="m2")
                            for a in (0, 32, 64, 96):
                                sw = a ^ 32
                                nc.vector.tensor_mul(m2[a:a + 32, :],
                                                     ps[sw:sw + 32, :],
                                                     ssg_sb[a:a + 32, tsl])
                            nc.vector.tensor_add(dstT[p][:, tsl], m1, m2)
                    # k content (from latent)
                    for h in range(NH):
                        ps = p1ps.tile([128, SL], f32, tag="proj", name="proj")
                        for g in range(4):
                            nc.tensor.matmul(ps, wuk_sb[g][:, h * DH:(h + 1) * DH],
                                             ckv[g], start=(g == 0), stop=(g == 3))
                        nc.vector.tensor_copy(out=k_cT[h][:, tsl], in_=ps)
                    # v in [t, dh] layout
                    for u in range(4):
                        ps = p1ps.tile([128, NH * DH], f32, tag="proj", name="proj")
                        for g in range(4):
                            nc.tensor.matmul(ps, ckv[g][:, u * 128:(u + 1) * 128],
                                             wuv_sb[g], start=(g == 0), stop=(g == 3))
                        nc.vector.tensor_copy(out=v_t[s * 4 + u], in_=ps)
                    # dilated-key V rows for this slice (keys t0 + 64j)
                    psA = vAps.tile([8, NH * DH], f32, tag="vA", name="vA")
                    for g in range(4):
                        nc.tensor.matmul(psA, ckv[g][:, 0:SL:STRIDE], wuv_sb[g],
                                         start=(g == 0), stop=(g == 3))
                    # engines can't write at non-32-aligned partition bases:
                    # stage at base 0, then SBUF->SBUF DMA into v_A rows
                    vst = rope_t.tile([8, NH * DH], bf, tag="vst", name="vst")
                    nc.vector.tensor_copy(out=vst, in_=psA)
                    nc.sync.dma_start(out=v_A[8 * s:8 * (s + 1), :], in_=vst)

            # ---------------- P2: block-sparse attention ----------------------
            with tc.tile_pool(name="wo", bufs=1) as wo:
              w_o_sb = [wo.tile([128, D], bf, tag=f"wo{h}", name=f"wo{h}") for h in range(NH)]
              for h in range(NH):
                  nc.sync.dma_start(out=w_o_sb[h],
                                    in_=w_o[h * DH:(h + 1) * DH, :])

              with tc.tile_pool(name="mk", bufs=2) as mk, \
                   tc.tile_pool(name="exp", bufs=14) as expp, \
                   tc.tile_pool(name="p2t", bufs=3) as p2t, \
                   tc.tile_pool(name="scps", bufs=3, space="PSUM") as scps, \
                   tc.tile_pool(name="dnps", bufs=2, space="PSUM") as dnps, \
                   tc.tile_pool(name="aops", bufs=2, space="PSUM") as aops:
                for qb in range(NB):
                    q0 = qb * QB
                    qsl = slice(q0, q0 + QB)
                    blk = [t_ for t_ in _MASK_TILES[qb] if t_["cls"] != "skip"]
                    msk = {}
                    for t_ in blk:
                        if t_["cls"] != "mask":
                            continue
                        rows = t_["m"].shape[0]
                        mt = mk.tile([rows, QB], bf, tag=f"m{t_['kind']}{t_['i']}", name=f"m{t_['kind']}{t_['i']}")
                        if t_["kind"] == "A0":
                            nc.sync.dma_start(out=mt, in_=mA0)
                        elif t_["kind"] == "A1":
                            nc.sync.dma_start(out=mt, in_=mA1[qb])
                        else:
                            nc.sync.dma_start(out=mt, in_=mB[qb, t_["i"]])
                        msk[(t_["kind"], t_["i"])] = mt
                    for h in range(NH):
                        pr, po = h // 2, (h % 2) * 64
                        q_c_sl = q_cT[h][:, qsl]
                        q_r_sl = q_rT[pr][po:po + 64, qsl]
                        exps = []
                        for t_ in blk:
                            kind, i, k0 = t_["kind"], t_["i"], t_["k0"]
                            rows = t_["m"].shape[0]
                            if kind == "A0":
                                lk = k_cT[h][:, 0:GLOB]
                                lr = k_rT[pr][po:po + 64, 0:GLOB]
                            elif kind == "A1":
                                lk = k_cT[h][:, 0:T:STRIDE]
                                lr = k_rT[pr][po:po + 64, 0:T:STRIDE]
                            else:
                                ks = slice(k0 + 128 * i, k0 + 128 * (i + 1))
                                lk = k_cT[h][:, ks]
                                lr = k_rT[pr][po:po + 64, ks]
                            ps = scps.tile([128, QB], f32, tag="sc", name="sc")
                            nc.tensor.matmul(ps[0:rows, :], lk, q_c_sl,
                                             start=True, stop=False)
                            nc.tensor.matmul(ps[0:rows, :], lr, q_r_sl,
                                             start=False, stop=True)
                            ex = expp.tile([128, QB], bf, tag="ex", name="ex")
                            nc.scalar.activation(out=ex[0:rows, :],
                                                 in_=ps[0:rows, :], func=EXP)
                            if t_["cls"] == "mask":
                                nc.vector.tensor_mul(ex[0:rows, :], ex[0:rows, :],
                                                     msk[(kind, i)])
                            exps.append((t_, rows, ex))
                        dn = dnps.tile([128, QB], f32, tag="dn", name="dn")
                        last = len(exps) - 1
                        for j, (t_, rows, ex) in enumerate(exps):
                            nc.tensor.matmul(dn, ones[0:rows, :], ex[0:rows, :],
                                             start=(j == 0), stop=(j == last))
                        rc = p2t.tile([128, QB], f32, tag="rc", name="rc")
                        nc.vector.reciprocal(out=rc, in_=dn)
                        ao = aops.tile([128, QB], f32, tag="ao", name="ao")
                        hs = slice(h * DH, (h + 1) * DH)
                        for j, (t_, rows, ex) in enumerate(exps):
                            kind, i, k0 = t_["kind"], t_["i"], t_["k0"]
                            if kind == "A0":
                                lv = v_t[0][:, hs]
                            elif kind == "A1":
                                lv = v_A[:, hs]
                            else:
                                lv = v_t[k0 // 128 + i][:, hs]
                            nc.tensor.matmul(ao, lv[0:rows, :], ex[0:rows, :],
                                             start=(j == 0), stop=(j == last))
                        nc.vector.tensor_mul(att[h][:, qsl], ao, rc)

              # ---------------- P3: output projection (row-parallel) --------
              with tc.tile_pool(name="p3s", bufs=4) as p3s, \
                   tc.tile_pool(name="p3ps", bufs=4, space="PSUM") as p3ps:
                for tt in range(NT):
                    tsl = slice(128 * tt, 128 * (tt + 1))
                    for dc in range(4):
                        csl = slice(512 * dc, 512 * (dc + 1))
                        ps = p3ps.tile([128, 512], f32, tag="o", name="o")
                        for h in range(NH):
                            nc.tensor.matmul(ps, att[h][:, tsl],
                                             w_o_sb[h][:, csl],
                                             start=(h == 0), stop=(h == 3))
                        ob = p3s.tile([128, 512], f32, tag="ob", name="ob")
                        nc.vector.tensor_copy(out=ob, in_=ps)
                        nc.sync.dma_start(out=out[tsl, csl], in_=ob)
    nc.compile()
    return nc


_NC = None


def _get_nc():
    global _NC
    if _NC is None:
        _NC = _build_program()
    return _NC


def _prep_in_maps(inputs):
    x = np.asarray(inputs["x"], np.float32)
    w_q = np.asarray(inputs["w_q"], np.float32)
    w_dkv = np.asarray(inputs["w_dkv"], np.float32)
    w_uk = np.asarray(inputs["w_uk"], np.float32)
    w_uv = np.asarray(inputs["w_uv"], np.float32)
    w_qp = np.asarray(inputs["w_q_pos"], np.float32)
    w_kp = np.asarray(inputs["w_k_pos"], np.float32)
    w_o = np.asarray(inputs["w_o"], np.float32)

    invf = _inv_freq()                                # [32]
    t = np.arange(T, dtype=np.float32)
    ang = t[None, :] * invf[:, None]                  # [32, T]
    cos32 = np.cos(ang)
    sin32 = np.sin(ang)
    cosT = np.tile(cos32, (4, 1)).astype(np.float32)  # rows p: f = p % 32
    ssgn = np.tile(sin32, (4, 1)).astype(np.float32)
    ssgn[0:32] *= -1.0
    ssgn[64:96] *= -1.0

    mA0 = _MASK_TILES[0][0]["m"].astype(np.float32).astype(BF16)
    mA1 = np.stack([_MASK_TILES[qb][1]["m"] for qb in range(NB)]) \
        .astype(np.float32).astype(BF16)
    mB = np.stack([[_MASK_TILES[qb][2 + i]["m"] for i in range(4)]
                   for qb in range(NB)]).astype(np.float32).astype(BF16)

    xT_b = [np.ascontiguousarray(x[b].T).astype(BF16) for b in range(B)]
    common = dict(cosT=cosT, ssgnT=ssgn, mA0=mA0, mA1=mA1, mB=mB,
                  w_dkv=w_dkv.astype(BF16))

    in_maps = []
    for c in range(NCORES):
        b, g = c // 4, c % 4
        ch = slice(4 * g * DH, 4 * (g + 1) * DH)      # content head cols / w_o rows
        rh = slice(4 * g * DR, 4 * (g + 1) * DR)      # rope head cols
        in_maps.append(dict(
            common,
            xT=xT_b[b],
            w_q=(w_q[:, ch] * SCALE).astype(BF16),
            w_uk=np.ascontiguousarray(w_uk[:, ch]).astype(BF16),
            w_uv=np.ascontiguousarray(w_uv[:, ch]).astype(BF16),
            w_qp=(w_qp[:, rh] * (SCALE_ROPE * YARN * YARN)).astype(BF16),
            w_kp=np.ascontiguousarray(w_kp[:, rh]).astype(BF16),
            w_o=np.ascontiguousarray(w_o[ch, :]).astype(BF16),
        ))
    return in_maps


def _run(inputs, trace=False, trace_kwargs=None):
    nc = _get_nc()
    in_maps = _prep_in_maps(inputs)
    res = bass_utils.run_bass_kernel_spmd(
        nc, in_maps, core_ids=list(range(NCORES)), trace=trace,
        **(trace_kwargs or {}))
    out = np.zeros((B, T, D), np.float32)
    for c in range(NCORES):
        out[c // 4] += res.results[c]["out"]
    return out, res


def kernel(**inputs) -> np.ndarray:
    out, _ = _run(inputs)
    return out


# revision 23
# speedup vs baseline: 1.4482x; 1.4482x over previous
"""DeepSeek sparse attention (MLA + YaRN RoPE + local/dilated/global mask) on 8 TRN2 cores.

Sharding: (batch, head-group) across 8 cores — core c handles batch c//4, heads
[4*(c%4), 4*(c%4)+4).  Each core computes its projections from the full x (host
pre-transposes x per batch), runs block-sparse attention for its 4 heads, and
produces a row-parallel partial of out @ w_o.  Host sums the 4 partials per batch.

Layout: "transposed" activations [feature, t] so every matmul keeps the moving
operand in the free dim (N=512/256) at full bf16 rate and no on-chip transposes
are needed anywhere.  Scores are computed as S^T[k, q]; the softmax denominator
is obtained with an all-ones stationary operand (broadcast across partitions),
so the divide is a plain elementwise mul by the reciprocal.
"""

import sys

if "/opt/trn_rl_repo" not in sys.path:
    sys.path.insert(0, "/opt/trn_rl_repo")

import ml_dtypes
import numpy as np

import concourse.bass as bass  # noqa: F401  (bass types used via tile/bacc)
import concourse.mybir as mybir
import concourse.tile as tile
from concourse import bacc, bass_utils

BF16 = ml_dtypes.bfloat16

# ---- problem constants (hardcoded per contract) ----
B, T, D = 2, 2048, 2048
H, DH, DR, DL = 16, 128, 64, 512
WINDOW, STRIDE, GLOB = 512, 64, 128
BASE, MAX_SEQ, ORIG_MAX = 10000.0, 131072, 4096
BETA_FAST, BETA_SLOW = 32.0, 1.0
SCALE = 1.0 / float(np.sqrt(DH))
SCALE_ROPE = 1.0 / float(np.sqrt(DR))
YARN = float(np.float32(0.1 * np.log(MAX_SEQ / ORIG_MAX) + 1.0))
HALF = WINDOW // 2

NCORES = 8
NH = 4            # heads per core
NP = 2            # head-pairs per core (rope tiles stack 2 heads on 128 partitions)
QB = 512          # query block
NB = T // QB      # 4
NBW = (HALF + QB) // 128   # window-strip tiles per block
SL = 512          # t-slice width in projection phase
NS = T // SL      # 4
NT = T // 128     # 16


def _inv_freq():
    base_inv = 1.0 / (BASE ** (np.arange(0, DR, 2, dtype=np.float32) / DR))
    scale = MAX_SEQ / ORIG_MAX
    freqs = np.arange(DR // 2, dtype=np.float32)
    ramp = np.clip((freqs - BETA_SLOW) / (BETA_FAST - BETA_SLOW), 0.0, 1.0)
    return (base_inv * (1 - ramp) + (base_inv / scale) * ramp).astype(np.float32)


def _full_mask():
    pos = np.arange(T)
    qp, kp = pos[:, None], pos[None, :]
    dist = qp - kp
    window = (dist >= -HALF) & (dist <= HALF)
    dil = (kp % STRIDE == 0) | (kp < GLOB)
    return (window | dil) & (kp <= qp)


def _mask_tiles():
    """Per q-block key tiles, with exactly-once ownership masks.

    Tiles: A0 = keys [0, 128) (global), A1 = 32 dilated keys {64j}, B0..B3 =
    the 512-wide sliding window strip.  A0 owns k<128; A1 owns k%64==0 & k>=128;
    B owns the rest.  Each tile classified: 'skip' (all-zero), 'ones', 'mask'.
    """
    full = _full_mask()
    blocks = []
    for qb in range(NB):
        q0 = qb * QB
        k0 = max(0, q0 - HALF)
        qs = slice(q0, q0 + QB)
        blk = []
        m = full[qs, 0:GLOB].T.copy()                        # [128, QB]
        blk.append(dict(kind="A0", i=0, k0=0, keys=np.arange(GLOB), m=m))
        keys = np.arange(32) * STRIDE
        m = full[qs, :][:, keys].T.copy()                    # [32, QB]
        m[keys < GLOB] = False
        blk.append(dict(kind="A1", i=0, k0=0, keys=keys, m=m))
        for i in range(NBW):
            kk = k0 + 128 * i + np.arange(128)
            m = full[qs, :][:, kk].T.copy()
            m[(kk < GLOB) | (kk % STRIDE == 0)] = False
            blk.append(dict(kind="B", i=i, k0=k0, keys=kk, m=m))
        for t_ in blk:
            t_["cls"] = ("skip" if not t_["m"].any()
                         else "ones" if t_["m"].all() else "mask")
            if t_["cls"] == "skip":
                t_["qr"] = (0, QB)
            else:
                cols = np.flatnonzero(t_["m"].any(axis=0))
                a, b = int(cols[0]), int(cols[-1]) + 1
                assert (t_["m"].any(axis=0)[a:b]).all()  # contiguous
                t_["qr"] = (a, b)
        # accumulation groups start with tile 0: it must span all q columns
        assert blk[0]["qr"] == (0, QB)
        blocks.append(blk)
    # exactly-once coverage check against the reference mask
    for qb in range(NB):
        cov = np.zeros((QB, T), dtype=np.int32)
        for t_ in blocks[qb]:
            cov[np.arange(QB)[:, None], t_["keys"][None, :]] += t_["m"].T
        assert (cov == full[qb * QB:(qb + 1) * QB].astype(np.int32)).all()
    return blocks


_MASK_TILES = _mask_tiles()


def _build_program():
    nc = bacc.Bacc("TRN2", target_bir_lowering=False, debug=False,
                   enable_asserts=False, num_devices=NCORES)
    bf, f32 = mybir.dt.bfloat16, mybir.dt.float32

    xT = nc.dram_tensor("xT", [D, T], bf, kind="ExternalInput").ap()
    w_q = nc.dram_tensor("w_q", [D, NH * DH], bf, kind="ExternalInput").ap()
    w_dkv = nc.dram_tensor("w_dkv", [D, DL], bf, kind="ExternalInput").ap()
    w_uk = nc.dram_tensor("w_uk", [DL, NH * DH], bf, kind="ExternalInput").ap()
    w_uv = nc.dram_tensor("w_uv", [DL, NH * DH], bf, kind="ExternalInput").ap()
    w_qp = nc.dram_tensor("w_qp", [D, NH * DR], bf, kind="ExternalInput").ap()
    w_kp = nc.dram_tensor("w_kp", [D, NH * DR], bf, kind="ExternalInput").ap()
    w_o = nc.dram_tensor("w_o", [NH * DH, D], bf, kind="ExternalInput").ap()
    cosT = nc.dram_tensor("cosT", [128, T], f32, kind="ExternalInput").ap()
    ssgnT = nc.dram_tensor("ssgnT", [128, T], f32, kind="ExternalInput").ap()
    mA0 = nc.dram_tensor("mA0", [GLOB, QB], bf, kind="ExternalInput").ap()
    mA1 = nc.dram_tensor("mA1", [NB, 32, QB], bf, kind="ExternalInput").ap()
    mB = nc.dram_tensor("mB", [NB, NBW, 128, QB], bf, kind="ExternalInput").ap()
    out = nc.dram_tensor("out", [T, D], f32, kind="ExternalOutput").ap()

    EXP = mybir.ActivationFunctionType.Exp

    with tile.TileContext(nc) as tc:
        with tc.tile_pool(name="acts", bufs=1) as acts, \
             tc.tile_pool(name="consts", bufs=1) as consts:
            q_cT = [acts.tile([128, T], bf, tag=f"q_cT{h}", name=f"q_cT{h}") for h in range(NH)]
            k_cT = [acts.tile([128, T], bf, tag=f"k_cT{h}", name=f"k_cT{h}") for h in range(NH)]
            q_rT = [acts.tile([128, T], bf, tag=f"q_rT{p}", name=f"q_rT{p}") for p in range(NP)]
            k_rT = [acts.tile([128, T], bf, tag=f"k_rT{p}", name=f"k_rT{p}") for p in range(NP)]
            v_t = [acts.tile([128, NH * DH], bf, tag=f"v{t_}", name=f"v{t_}") for t_ in range(NT)]
            v_A = acts.tile([32, NH * DH], bf, tag="v_A", name="v_A")
            att = [acts.tile([128, T], bf, tag=f"att{h}", name=f"att{h}") for h in range(NH)]
            ones = consts.tile([128, 128], bf, tag="ones", name="ones")
            nc.vector.memset(ones, 1.0)
            # load the Exp LUT into ACT early: the first real exp otherwise
            # pays a 1.3us table load right when P2's PSUM rotation is tight
            warm = consts.tile([1, 2], f32, tag="warm", name="warm")
            nc.vector.memset(warm, 0.0)
            nc.scalar.activation(out=warm, in_=warm, func=EXP)

            # ---------------- P1: projections (t-slice streamed) --------------
            with tc.tile_pool(name="wp1", bufs=1) as wp1, \
                 tc.tile_pool(name="xs", bufs=2) as xs, \
                 tc.tile_pool(name="ckvp", bufs=2) as ckvp, \
                 tc.tile_pool(name="rope_t", bufs=3) as rope_t, \
                 tc.tile_pool(name="p1ps", bufs=6, space="PSUM") as p1ps, \
                 tc.tile_pool(name="vAps", bufs=2, space="PSUM") as vAps:
                # one big tile per weight tensor -> one DMA each; slice views
                # [p, d*cols + c] address D-tile d, col c
                wq_sb = wp1.tile([128, 16 * NH * DH], bf, tag="wq", name="wq")
                wdkv_sb = wp1.tile([128, 16 * DL], bf, tag="wdkv", name="wdkv")
                wqp_sb = wp1.tile([128, 16 * NH * DR], bf, tag="wqp", name="wqp")
                wkp_sb = wp1.tile([128, 16 * NH * DR], bf, tag="wkp", name="wkp")
                wuk_sb = wp1.tile([128, 4 * NH * DH], bf, tag="wuk", name="wuk")
                wuv_sb = wp1.tile([128, 4 * NH * DH], bf, tag="wuv", name="wuv")
                cos_sb = wp1.tile([128, T], f32, tag="cos", name="cos")
                ssg_sb = wp1.tile([128, T], f32, tag="ssg", name="ssg")

                def _wslice(big, cols, d, c0, c1):
                    return big[:, d * cols + c0:d * cols + c1]

                def _load_w(dst, src, cols):
                    nc.sync.dma_start(
                        out=dst.rearrange("p (n m) -> p n m", m=cols),
                        in_=src.rearrange("(n p) m -> p n m", p=128))

                # ordered so slice-0 compute can start ASAP; first loads are
                # split in 4 so the d=0 matmuls start after ~1/4 of the DMA
                xts_cur = xs.tile([128, 16 * SL], bf, tag="xts", name="xts")
                wq_r = w_q.rearrange("(n p) m -> p n m", p=128)
                xT_r = xT.rearrange("(n p) m -> p n m", p=128)
                for c in range(4):
                    cs = slice(4 * c, 4 * (c + 1))
                    nc.sync.dma_start(
                        out=wq_sb.rearrange("p (n m) -> p n m", m=NH * DH)[:, cs],
                        in_=wq_r[:, cs])
                    nc.sync.dma_start(
                        out=xts_cur.rearrange("p (n m) -> p n m", m=SL)[:, cs],
                        in_=xT_r[:, cs, 0:SL])
                _load_w(wdkv_sb, w_dkv, DL)
                _load_w(wqp_sb, w_qp, NH * DR)
                _load_w(wkp_sb, w_kp, NH * DR)
                nc.sync.dma_start(out=cos_sb, in_=cosT)
                nc.sync.dma_start(out=ssg_sb, in_=ssgnT)
                _load_w(wuk_sb, w_uk, NH * DH)
                _load_w(wuv_sb, w_uv, NH * DH)

                for s in range(NS):
                    t0 = s * SL
                    tsl = slice(t0, t0 + SL)
                    xts_nxt = None
                    if s + 1 < NS:  # prefetch next slice into the other slot
                        xts_nxt = xs.tile([128, 16 * SL], bf, tag="xts", name="xts")
                        nc.sync.dma_start(
                            out=xts_nxt.rearrange("p (n m) -> p n m", m=SL),
                            in_=xT.rearrange("(n p) m -> p n m", p=128)
                                [:, :, (s + 1) * SL:(s + 2) * SL])
                    xt = [xts_cur[:, d * SL:(d + 1) * SL] for d in range(16)]
                    # content q projections
                    for h in range(NH):
                        ps = p1ps.tile([128, SL], f32, tag="proj", name="proj")
                        for d in range(16):
                            nc.tensor.matmul(
                                ps, _wslice(wq_sb, NH * DH, d, h * DH, (h + 1) * DH),
                                xt[d], start=(d == 0), stop=(d == 15))
                        nc.vector.tensor_copy(out=q_cT[h][:, tsl], in_=ps)
                    # latent kv
                    ckv = [ckvp.tile([128, SL], bf, tag=f"ckv{g}", name=f"ckv{g}") for g in range(4)]
                    for g in range(4):
                        ps = p1ps.tile([128, SL], f32, tag="proj", name="proj")
                        for d in range(16):
                            nc.tensor.matmul(
                                ps, _wslice(wdkv_sb, DL, d, g * 128, (g + 1) * 128),
                                xt[d], start=(d == 0), stop=(d == 15))
                        nc.vector.tensor_copy(out=ckv[g], in_=ps)
                    # rope projections + rotation (pair-stacked: 2 heads / tile)
                    for w_sb, dstT in ((wqp_sb, q_rT), (wkp_sb, k_rT)):
                        for p in range(NP):
                            ps = p1ps.tile([128, SL], f32, tag="proj", name="proj")
                            for d in range(16):
                                nc.tensor.matmul(
                                    ps, _wslice(w_sb, NH * DR, d, p * 128, (p + 1) * 128),
                                    xt[d], start=(d == 0), stop=(d == 15))
                            m1 = rope_t.tile([128, SL], bf, tag="m1", name="m1")
                            nc.vector.tensor_mul(m1, ps, cos_sb[:, tsl])
                            m2 = rope_t.tile([128, SL], bf, tag="m2", name="m2")
                            for a in (0, 32, 64, 96):
                                sw = a ^ 32
                                nc.vector.tensor_mul(m2[a:a + 32, :],
                                                     ps[sw:sw + 32, :],
                                                     ssg_sb[a:a + 32, tsl])
                            nc.vector.tensor_add(dstT[p][:, tsl], m1, m2)
                    # k content (from latent)
                    for h in range(NH):
                        ps = p1ps.tile([128, SL], f32, tag="proj", name="proj")
                        for g in range(4):
                            nc.tensor.matmul(
                                ps, _wslice(wuk_sb, NH * DH, g, h * DH, (h + 1) * DH),
                                ckv[g], start=(g == 0), stop=(g == 3))
                        nc.vector.tensor_copy(out=k_cT[h][:, tsl], in_=ps)
                    # v in [t, dh] layout
                    for u in range(4):
                        ps = p1ps.tile([128, NH * DH], f32, tag="proj", name="proj")
                        for g in range(4):
                            nc.tensor.matmul(
                                ps, ckv[g][:, u * 128:(u + 1) * 128],
                                wuv_sb[:, g * NH * DH:(g + 1) * NH * DH],
                                start=(g == 0), stop=(g == 3))
                        nc.vector.tensor_copy(out=v_t[s * 4 + u], in_=ps)
                    # dilated-key V rows for this slice (keys t0 + 64j)
                    psA = vAps.tile([8, NH * DH], f32, tag="vA", name="vA")
                    for g in range(4):
                        nc.tensor.matmul(psA, ckv[g][:, 0:SL:STRIDE],
                                         wuv_sb[:, g * NH * DH:(g + 1) * NH * DH],
                                         start=(g == 0), stop=(g == 3))
                    # engines can't write at non-32-aligned partition bases:
                    # stage at base 0, then SBUF->SBUF DMA into v_A rows
                    vst = rope_t.tile([8, NH * DH], bf, tag="vst", name="vst")
                    nc.vector.tensor_copy(out=vst, in_=psA)
                    nc.sync.dma_start(out=v_A[8 * s:8 * (s + 1), :], in_=vst)
                    xts_cur = xts_nxt

            # ---------------- P2: block-sparse attention ----------------------
            with tc.tile_pool(name="wo", bufs=1) as wo:
              w_o_sb = wo.tile([128, NH * D], bf, tag="wo", name="wo")
              nc.sync.dma_start(
                  out=w_o_sb.rearrange("p (n m) -> p n m", m=D),
                  in_=w_o.rearrange("(n p) m -> p n m", p=128))

              with tc.tile_pool(name="mk", bufs=2) as mk, \
                   tc.tile_pool(name="exp", bufs=48) as expp, \
                   tc.tile_pool(name="p2t", bufs=3) as p2t, \
                   tc.tile_pool(name="p3s", bufs=2) as p3s, \
                   tc.tile_pool(name="scps", bufs=4, space="PSUM") as scps, \
                   tc.tile_pool(name="daops", bufs=2, space="PSUM") as daops, \
                   tc.tile_pool(name="p3ps", bufs=2, space="PSUM") as p3ps:
                rcs = {}

                def p2_chunks(qb, exps):
                    """pass 2 of block qb as deferred emitters (2 per head)."""
                    q0 = qb * QB
                    qsl = slice(q0, q0 + QB)
                    blk = [t_ for t_ in _MASK_TILES[qb] if t_["cls"] != "skip"]
                    last = len(blk) - 1

                    def dn_chunk(h):
                        dn = daops.tile([128, QB], f32, tag="dao", name="dn")
                        for j, t_ in enumerate(blk):
                            rows, ex = exps[(h, t_["kind"], t_["i"])]
                            a, b = t_["qr"]
                            nc.tensor.matmul(dn[:, a:b], ones[0:rows, :],
                                             ex[0:rows, 0:b - a],
                                             start=(j == 0), stop=(j == last))
                        rc = p2t.tile([128, QB], f32, tag="rc", name="rc")
                        nc.vector.reciprocal_approx_fast(out=rc, in_=dn)
                        rcs[(qb, h)] = rc

                    def ao_chunk(h):
                        hs = slice(h * DH, (h + 1) * DH)
                        ao = daops.tile([128, QB], f32, tag="dao", name="ao")
                        for j, t_ in enumerate(blk):
                            kind, i, k0 = t_["kind"], t_["i"], t_["k0"]
                            rows, ex = exps[(h, kind, i)]
                            if kind == "A0":
                                lv = v_t[0][:, hs]
                            elif kind == "A1":
                                lv = v_A[:, hs]
                            else:
                                lv = v_t[k0 // 128 + i][:, hs]
                            a, b = t_["qr"]
                            nc.tensor.matmul(ao[:, a:b], lv[0:rows, :],
                                             ex[0:rows, 0:b - a],
                                             start=(j == 0), stop=(j == last))
                        nc.vector.tensor_mul(att[h][:, qsl],
                                             ao, rcs.pop((qb, h)))

                    out_c = []
                    for h in range(NH):
                        out_c.append(lambda h=h: dn_chunk(h))
                        out_c.append(lambda h=h: ao_chunk(h))
                    return out_c

                def p3_chunk(tt):
                    """output projection for one 128-row t-tile."""
                    def run():
                        tsl = slice(128 * tt, 128 * (tt + 1))
                        ob = p3s.tile([128, D], f32, tag="ob", name="ob")
                        for dc in range(4):
                            csl = slice(512 * dc, 512 * (dc + 1))
                            ps = p3ps.tile([128, 512], f32, tag="o", name="o")
                            for h in range(NH):
                                nc.tensor.matmul(ps, att[h][:, tsl],
                                                 w_o_sb[:, h * D + 512 * dc:
                                                        h * D + 512 * (dc + 1)],
                                                 start=(h == 0), stop=(h == 3))
                            nc.vector.tensor_copy(out=ob[:, csl], in_=ps)
                        nc.sync.dma_start(out=out[tsl, :], in_=ob)
                    return run

                work = []
                for qb in range(NB):
                    q0 = qb * QB
                    qsl = slice(q0, q0 + QB)
                    # A1 depends on the strided full-T k tensors (last P1
                    # slice); emit it last so early score matmuls don't stall.
                    blk = [t_ for t_ in _MASK_TILES[qb] if t_["cls"] != "skip"]
                    blk = ([t_ for t_ in blk if t_["kind"] != "A1"]
                           + [t_ for t_ in blk if t_["kind"] == "A1"])
                    msk = {}
                    if any(t_["kind"] == "B" and t_["cls"] == "mask" for t_ in blk):
                        mbs = mk.tile([128, NBW * QB], bf, tag="mbs", name="mbs")
                        nc.sync.dma_start(
                            out=mbs.rearrange("p (n m) -> p n m", m=QB),
                            in_=mB[qb].rearrange("n p m -> p n m"))
                        for t_ in blk:
                            if t_["kind"] == "B" and t_["cls"] == "mask":
                                msk[("B", t_["i"])] = mbs[:, t_["i"] * QB:
                                                          (t_["i"] + 1) * QB]
                    for t_ in blk:
                        if t_["cls"] != "mask" or t_["kind"] == "B":
                            continue
                        rows = t_["m"].shape[0]
                        mt = mk.tile([rows, QB], bf, tag=f"m{t_['kind']}",
                                     name=f"m{t_['kind']}")
                        nc.sync.dma_start(
                            out=mt, in_=mA0 if t_["kind"] == "A0" else mA1[qb])
                        msk[(t_["kind"], t_["i"])] = mt
                    # pass 1: scores + exp + mask, all heads, one key tile at
                    # a time; deferred pass-2/P3 chunks of earlier blocks are
                    # interleaved to keep the PE busy while ACT runs the exps.
                    # Rope matmuls of a head pair are adjacent: disjoint row
                    # groups run concurrently on the PE.
                    exps = {}
                    for t_ in blk:
                        kind, i, k0 = t_["kind"], t_["i"], t_["k0"]
                        rows = t_["m"].shape[0]
                        a, b = t_["qr"]
                        w = b - a
                        qv = slice(q0 + a, q0 + b)
                        pss = []
                        for h in range(NH):
                            if kind == "A0":
                                lk = k_cT[h][:, 0:GLOB]
                            elif kind == "A1":
                                lk = k_cT[h][:, 0:T:STRIDE]
                            else:
                                ks = slice(k0 + 128 * i, k0 + 128 * (i + 1))
                                lk = k_cT[h][:, ks]
                            ps = scps.tile([128, QB], f32, tag="sc", name="sc")
                            nc.tensor.matmul(ps[0:rows, 0:w], lk, q_cT[h][:, qv],
                                             start=True, stop=False)
                            pss.append(ps)
                        for h in range(NH):
                            pr, po = h // 2, (h % 2) * 64
                            if kind == "A0":
                                lr = k_rT[pr][po:po + 64, 0:GLOB]
                            elif kind == "A1":
                                lr = k_rT[pr][po:po + 64, 0:T:STRIDE]
                            else:
                                ks = slice(k0 + 128 * i, k0 + 128 * (i + 1))
                                lr = k_rT[pr][po:po + 64, ks]
                            nc.tensor.matmul(pss[h][0:rows, 0:w], lr,
                                             q_rT[pr][po:po + 64, qv],
                                             start=False, stop=True)
                        for h in range(NH):
                            ex = expp.tile([128, QB], bf, tag="ex", name="ex")
                            nc.scalar.activation(out=ex[0:rows, 0:w],
                                                 in_=pss[h][0:rows, 0:w], func=EXP)
                            if t_["cls"] == "mask":
                                nc.vector.tensor_mul(ex[0:rows, 0:w],
                                                     ex[0:rows, 0:w],
                                                     msk[(kind, i)][:, a:b])
                            exps[(h, kind, i)] = (rows, ex)
                        for _ in range(2 if len(work) > 8 else 1):
                            if work:
                                work.pop(0)()
                    work.extend(p2_chunks(qb, exps))
                    if qb >= 1:
                        work.extend(p3_chunk(tt) for tt in
                                    range(4 * (qb - 1), 4 * qb))
                for ch in work:
                    ch()
                for tt in range(4 * (NB - 1), 4 * NB):
                    p3_chunk(tt)()
    nc.compile()
    return nc


_NC = None


def _get_nc():
    global _NC
    if _NC is None:
        _NC = _build_program()
    return _NC


def _prep_in_maps(inputs):
    x = np.asarray(inputs["x"], np.float32)
    w_q = np.asarray(inputs["w_q"], np.float32)
    w_dkv = np.asarray(inputs["w_dkv"], np.float32)
    w_uk = np.asarray(inputs["w_uk"], np.float32)
    w_uv = np.asarray(inputs["w_uv"], np.float32)
    w_qp = np.asarray(inputs["w_q_pos"], np.float32)
    w_kp = np.asarray(inputs["w_k_pos"], np.float32)
    w_o = np.asarray(inputs["w_o"], np.float32)

    invf = _inv_freq()                                # [32]
    t = np.arange(T, dtype=np.float32)
    ang = t[None, :] * invf[:, None]                  # [32, T]
    cos32 = np.cos(ang)
    sin32 = np.sin(ang)
    cosT = np.tile(cos32, (4, 1)).astype(np.float32)  # rows p: f = p % 32
    ssgn = np.tile(sin32, (4, 1)).astype(np.float32)
    ssgn[0:32] *= -1.0
    ssgn[64:96] *= -1.0

    mA0 = _MASK_TILES[0][0]["m"].astype(np.float32).astype(BF16)
    mA1 = np.stack([_MASK_TILES[qb][1]["m"] for qb in range(NB)]) \
        .astype(np.float32).astype(BF16)
    mB = np.stack([[_MASK_TILES[qb][2 + i]["m"] for i in range(NBW)]
                   for qb in range(NB)]).astype(np.float32).astype(BF16)

    xT_b = [np.ascontiguousarray(x[b].T).astype(BF16) for b in range(B)]
    common = dict(cosT=cosT, ssgnT=ssgn, mA0=mA0, mA1=mA1, mB=mB,
                  w_dkv=w_dkv.astype(BF16))

    in_maps = []
    for c in range(NCORES):
        b, g = c // 4, c % 4
        ch = slice(4 * g * DH, 4 * (g + 1) * DH)      # content head cols / w_o rows
        rh = slice(4 * g * DR, 4 * (g + 1) * DR)      # rope head cols
        in_maps.append(dict(
            common,
            xT=xT_b[b],
            w_q=(w_q[:, ch] * SCALE).astype(BF16),
            w_uk=np.ascontiguousarray(w_uk[:, ch]).astype(BF16),
            w_uv=np.ascontiguousarray(w_uv[:, ch]).astype(BF16),
            w_qp=(w_qp[:, rh] * (SCALE_ROPE * YARN * YARN)).astype(BF16),
            w_kp=np.ascontiguousarray(w_kp[:, rh]).astype(BF16),
            w_o=np.ascontiguousarray(w_o[ch, :]).astype(BF16),
        ))
    return in_maps


def _run(inputs, trace=False, trace_kwargs=None):
    nc = _get_nc()
    in_maps = _prep_in_maps(inputs)
    res = bass_utils.run_bass_kernel_spmd(
        nc, in_maps, core_ids=list(range(NCORES)), trace=trace,
        **(trace_kwargs or {}))
    out = np.zeros((B, T, D), np.float32)
    for c in range(NCORES):
        out[c // 4] += res.results[c]["out"]
    return out, res


def kernel(**inputs) -> np.ndarray:
    out, _ = _run(inputs)
    return out
